# revision 6
# baseline (speedup 1.0000x reference)
"""LSTM (B=4096, T=128, D=78, H=32) + Linear(32->2) on 8 NeuronCores.

Data-parallel over batch: 512 batch rows per core. On-core layout is
batch-on-partition (4 chunks of 128), gate values packed along the free
dimension so every elementwise op uses all 128 lanes. fp16 data / weights,
fp32 cell state and PSUM accumulation.
"""

import sys

sys.path.insert(0, "/opt/trn_rl_repo")

import numpy as np

B, T, D, H = 4096, 128, 78, 32
NCORES = 8
BC = B // NCORES          # 512 batch per core
NCHUNK = BC // 128        # 4 chunks of 128 batch
NSTREAM = 2               # independent pipelined streams
CPS = NCHUNK // NSTREAM   # chunks per stream

_CACHE = {}


def _build_program():
    import concourse.bacc as bacc
    import concourse.bass as bass
    import concourse.tile as tile
    from concourse import mybir
    from contextlib import ExitStack

    f16 = mybir.dt.float16
    f32 = mybir.dt.float32
    Sigmoid = mybir.ActivationFunctionType.Sigmoid
    Tanh = mybir.ActivationFunctionType.Tanh

    nc = bacc.Bacc("TRN2", target_bir_lowering=False, debug=False)

    xT_d = nc.dram_tensor("xT", [T, D + 1, BC], f16, kind="ExternalInput")
    wih_d = nc.dram_tensor("wih", [D + 1, 128], f16, kind="ExternalInput")
    whh4_d = nc.dram_tensor("whh4", [128, 128], f16, kind="ExternalInput")
    woutb_d = nc.dram_tensor("woutb", [128, CPS, 2, H], f16, kind="ExternalInput")
    bout_d = nc.dram_tensor("bout", [128, NCHUNK, 2], f32, kind="ExternalInput")
    out_d = nc.dram_tensor("out", [128, NCHUNK, 2], f32, kind="ExternalOutput")

    with ExitStack() as ctx:
        tc = ctx.enter_context(tile.TileContext(nc))
        const = ctx.enter_context(tc.tile_pool(name="const", bufs=1))
        xbufs = ctx.enter_context(tc.tile_pool(name="xbufs", bufs=8))
        psum = ctx.enter_context(tc.tile_pool(name="psum", bufs=2, space="PSUM"))
        work = ctx.enter_context(tc.tile_pool(name="work", bufs=3))
        state = ctx.enter_context(tc.tile_pool(name="state", bufs=1))

        wih_sb = const.tile([D + 1, 128], f16)
        nc.default_dma_engine.dma_start(out=wih_sb[:], in_=wih_d.ap())
        whh4_sb = const.tile([128, 128], f16)
        nc.default_dma_engine.dma_start(out=whh4_sb[:], in_=whh4_d.ap())
        woutb_sb = const.tile([128, CPS, 2, H], f16)
        nc.default_dma_engine.dma_start(out=woutb_sb[:], in_=woutb_d.ap())
        bout_sb = const.tile([128, NCHUNK, 2], f32)
        nc.default_dma_engine.dma_start(out=bout_sb[:], in_=bout_d.ap())

        # persistent per-stream cell state [128, CPS, 32] fp32
        c_st = [
            state.tile([128, CPS, H], f32, tag=f"c{s}", name=f"c{s}")
            for s in range(NSTREAM)
        ]
        hT = [None] * NSTREAM
        hh_last = [None] * NSTREAM

        xT_ap = xT_d.ap()

        for t in range(T):
            x_sb = xbufs.tile([D + 1, BC], f16, tag="x")
            nc.default_dma_engine.dma_start(out=x_sb[:], in_=xT_ap[t])

            for s in range(NSTREAM):
                G = psum.tile([128, CPS, 512], f32, tag=f"G{s}")
                # per chunk: input contribution (+bias via ones row) opens the
                # accumulation group, the 32x32 recurrent tiles close it
                for jl in range(CPS):
                    j = s * CPS + jl
                    nc.tensor.matmul(
                        G[:, jl, 0:128],
                        x_sb[:, 128 * j : 128 * (j + 1)],
                        wih_sb[:],
                        start=True,
                        stop=True,
                    )
                    if t > 0:
                        for a in range(4):
                            nc.tensor.matmul(
                                G[32 * a : 32 * (a + 1), jl, 0:128],
                                hT[s][32 * a : 32 * (a + 1), H * jl : H * (jl + 1)],
                                whh4_sb[32 * a : 32 * (a + 1), :],
                                start=False,
                                stop=False,
                                skip_group_check=True,
                                tile_position=(32 * a, 32 * a),
                            )

                sifo = work.tile([128, CPS, 3 * H], f16, tag=f"sifo{s}")
                nc.scalar.activation(sifo[:], G[:, :, 0 : 3 * H], Sigmoid)
                gt = work.tile([128, CPS, H], f16, tag=f"g{s}")
                nc.scalar.activation(gt[:], G[:, :, 3 * H : 4 * H], Tanh)

                c = c_st[s]
                if t == 0:
                    nc.vector.tensor_mul(c[:], sifo[:, :, 0:H], gt[:])
                else:
                    u = work.tile([128, CPS, H], f16, tag=f"u{s}")
                    nc.vector.tensor_mul(u[:], sifo[:, :, 0:H], gt[:])
                    nc.vector.tensor_mul(c[:], sifo[:, :, H : 2 * H], c[:])
                    nc.vector.tensor_add(c[:], c[:], u[:])

                tct = work.tile([128, CPS, H], f16, tag=f"tc{s}")
                nc.scalar.activation(tct[:], c[:], Tanh)
                hh = work.tile([128, CPS, H], f16, tag=f"hh{s}")
                nc.vector.tensor_mul(hh[:], sifo[:, :, 2 * H : 3 * H], tct[:])
                if t == T - 1:
                    hh_last[s] = hh
                else:
                    hTn = work.tile([128, CPS * H], f16, tag=f"hT{s}")
                    nc.vector.transpose(hTn[:], hh[:])
                    hT[s] = hTn

        # final projection via DVE: out[p,j,o] = sum_h hh[p,j,h]*W_out[o,h]
        tmp_po = work.tile([128, NSTREAM, CPS, 2, H], f32, name="tmp_po", tag="tmp_po")
        out_raw = const.tile([128, NCHUNK, 2], f32, name="out_raw")
        for s in range(NSTREAM):
            for o in range(2):
                nc.vector.tensor_mul(
                    tmp_po[:, s, :, o, :], hh_last[s][:], woutb_sb[:, :, o, :]
                )
        nc.vector.tensor_reduce(
            out_raw[:], tmp_po[:], axis=mybir.AxisListType.X, op=mybir.AluOpType.add
        )
        nc.vector.tensor_add(out_raw[:], out_raw[:], bout_sb[:])
        nc.default_dma_engine.dma_start(out=out_d.ap(), in_=out_raw[:])

    nc.compile()
    return nc


def _prep_inputs(x, W_ih, W_hh, b_ih, b_hh, W_out, b_out):
    # reorder pytorch gate rows [i,f,g,o] -> [i,f,o,g] so sigmoid gates are
    # contiguous in the free dim
    perm = np.concatenate(
        [np.arange(0, H), np.arange(H, 2 * H), np.arange(3 * H, 4 * H),
         np.arange(2 * H, 3 * H)]
    )
    Wih_r = np.asarray(W_ih)[perm]          # [128, 78]
    Whh_r = np.asarray(W_hh)[perm]          # [128, 32]
    bias_r = (np.asarray(b_ih) + np.asarray(b_hh))[perm]  # [128]

    wih = np.concatenate([Wih_r.T, bias_r[None, :]], axis=0).astype(np.float16)
    whh4 = np.tile(Whh_r.T, (4, 1)).astype(np.float16)              # [128, 128]
    woutb = np.tile(
        np.asarray(W_out)[None, None], (128, CPS, 1, 1)
    ).astype(np.float16)  # [128, CPS, 2, 32]
    bout = np.tile(np.asarray(b_out)[None, None, :], (128, NCHUNK, 1)).astype(
        np.float32
    )

    # x: [B, T, D] -> [T, D, B] fp16 with ones row appended -> [T, 79, B]
    xf = np.asarray(x).astype(np.float16)
    xT = np.empty((T, D + 1, B), np.float16)
    xT[:, :D, :] = xf.transpose(1, 2, 0)
    xT[:, D, :] = np.float16(1.0)

    in_maps = []
    for c in range(NCORES):
        in_maps.append(
            {
                "xT": np.ascontiguousarray(xT[:, :, BC * c : BC * (c + 1)]),
                "wih": wih,
                "whh4": whh4,
                "woutb": woutb,
                "bout": bout,
            }
        )
    return in_maps


def kernel(x, W_ih, W_hh, b_ih, b_hh, W_out, b_out, _trace=False):
    from concourse.bass_utils import run_bass_kernel_spmd

    if "nc" not in _CACHE:
        _CACHE["nc"] = _build_program()
    nc = _CACHE["nc"]

    in_maps = _prep_inputs(x, W_ih, W_hh, b_ih, b_hh, W_out, b_out)
    res = run_bass_kernel_spmd(nc, in_maps, list(range(NCORES)), trace=_trace)
    _CACHE["last_result"] = res

    out = np.empty((B, 2), np.float32)
    for c in range(NCORES):
        oc = res.results[c]["out"]          # [128, 4, 2]
        out[BC * c : BC * (c + 1)] = oc.transpose(1, 0, 2).reshape(BC, 2)
    return out


if __name__ == "__main__":
    rng = np.random.default_rng(0)
    ins = {
        "x": rng.standard_normal((B, T, D), dtype=np.float32),
        "W_ih": rng.uniform(-0.18, 0.18, (4 * H, D)).astype(np.float32),
        "W_hh": rng.uniform(-0.18, 0.18, (4 * H, H)).astype(np.float32),
        "b_ih": rng.uniform(-0.18, 0.18, (4 * H,)).astype(np.float32),
        "b_hh": rng.uniform(-0.18, 0.18, (4 * H,)).astype(np.float32),
        "W_out": rng.uniform(-0.18, 0.18, (2, H)).astype(np.float32),
        "b_out": rng.uniform(-0.18, 0.18, (2,)).astype(np.float32),
    }
    o = kernel(**ins)
    print(o.shape, o[:4])


# revision 11
# speedup vs baseline: 1.0932x; 1.0932x over previous
"""LSTM (B=4096, T=128, D=78, H=32) + Linear(32->2) on 8 NeuronCores.

Data-parallel over batch: 512 batch rows per core. On-core layout is
batch-on-partition (4 chunks of 128), gate values packed along the free
dimension so every elementwise op uses all 128 lanes. fp16 data / weights,
fp32 cell state and PSUM accumulation.
"""

import sys

sys.path.insert(0, "/opt/trn_rl_repo")

import numpy as np

B, T, D, H = 4096, 128, 78, 32
import os as _os0
T = int(_os0.environ.get("K_T", "128"))
NCORES = 8
BC = B // NCORES          # 512 batch per core
NCHUNK = BC // 128        # 4 chunks of 128 batch
import os as _os
NSTREAM = int(_os.environ.get("K_NSTREAM", "2"))  # independent pipelined streams
CPS = NCHUNK // NSTREAM   # chunks per stream

_CACHE = {}


def _build_program():
    import concourse.bacc as bacc
    import concourse.bass as bass
    import concourse.tile as tile
    from concourse import mybir
    from contextlib import ExitStack

    f16 = mybir.dt.float16
    f32 = mybir.dt.float32
    Sigmoid = mybir.ActivationFunctionType.Sigmoid
    Tanh = mybir.ActivationFunctionType.Tanh

    nc = bacc.Bacc("TRN2", target_bir_lowering=False, debug=False)

    xT_d = nc.dram_tensor("xT", [T, D + 1, BC], f16, kind="ExternalInput")
    wih_d = nc.dram_tensor("wih", [D + 1, 128], f16, kind="ExternalInput")
    whh4_d = nc.dram_tensor("whh4", [128, 128], f16, kind="ExternalInput")
    woutb_d = nc.dram_tensor("woutb", [128, CPS, 2, H], f16, kind="ExternalInput")
    bout_d = nc.dram_tensor("bout", [128, NCHUNK, 2], f32, kind="ExternalInput")
    out_d = nc.dram_tensor("out", [128, NCHUNK, 2], f32, kind="ExternalOutput")

    with ExitStack() as ctx:
        tc = ctx.enter_context(tile.TileContext(nc))
        const = ctx.enter_context(tc.tile_pool(name="const", bufs=1))
        xbufs = ctx.enter_context(tc.tile_pool(name="xbufs", bufs=8))
        psum = ctx.enter_context(tc.tile_pool(name="psum", bufs=2, space="PSUM"))
        work = ctx.enter_context(tc.tile_pool(name="work", bufs=3))
        state = ctx.enter_context(tc.tile_pool(name="state", bufs=1))

        wih_sb = const.tile([D + 1, 128], f16)
        nc.default_dma_engine.dma_start(out=wih_sb[:], in_=wih_d.ap())
        whh4_sb = const.tile([128, 128], f16)
        nc.default_dma_engine.dma_start(out=whh4_sb[:], in_=whh4_d.ap())
        woutb_sb = const.tile([128, CPS, 2, H], f16)
        nc.default_dma_engine.dma_start(out=woutb_sb[:], in_=woutb_d.ap())
        bout_sb = const.tile([128, NCHUNK, 2], f32)
        nc.default_dma_engine.dma_start(out=bout_sb[:], in_=bout_d.ap())

        # persistent per-stream cell state [128, CPS, 32] fp32
        c_st = [
            state.tile([128, CPS, H], f32, tag=f"c{s}", name=f"c{s}")
            for s in range(NSTREAM)
        ]
        hT = [None] * NSTREAM
        hh_last = [None] * NSTREAM

        xT_ap = xT_d.ap()

        # Emit instructions in an explicitly staggered order: the Tile list
        # scheduler follows emission order as priority, so stream 1 is
        # offset by half a step to pipeline the serial per-step chain.
        sched = []

        def emit_step(s, t):
            def do_dma():
                x_sb = xbufs.tile([D + 1, BC], f16, tag="x", name=f"x{t}")
                x_tiles[t] = x_sb
                nc.default_dma_engine.dma_start(out=x_sb[:], in_=xT_ap[t])

            def do_mm():
                G = psum.tile([128, CPS, 512], f32, tag=f"G{s}", name=f"G{s}_{t}")
                G_t[s] = G
                for jl in range(CPS):
                    j = s * CPS + jl
                    nc.tensor.matmul(
                        G[:, jl, 0:128],
                        x_tiles[t][:, 128 * j : 128 * (j + 1)],
                        wih_sb[:],
                        start=True,
                        stop=True,
                    )
                    if t > 0:
                        for a in range(4):
                            nc.tensor.matmul(
                                G[32 * a : 32 * (a + 1), jl, 0:128],
                                hT[s][32 * a : 32 * (a + 1), H * jl : H * (jl + 1)],
                                whh4_sb[32 * a : 32 * (a + 1), :],
                                start=False,
                                stop=False,
                                skip_group_check=True,
                                tile_position=(32 * a, 32 * a),
                            )

            def do_sigma():
                G = G_t[s]
                sifo = work.tile([128, CPS, 4 * H], f16, tag=f"sifo{s}",
                                 name=f"sifo{s}_{t}")
                nc.scalar.activation(sifo[:], G[:, :, 0 : 4 * H], Sigmoid)
                sifo_t[s] = sifo

            def do_tanhg():
                # g was pre-scaled by 2 in the weights: tanh(x) = 2*sigmoid(2x)-1
                gt = work.tile([128, CPS, H], f16, tag=f"g{s}", name=f"g{s}_{t}")
                nc.vector.tensor_scalar(
                    gt[:], sifo_t[s][:, :, 3 * H : 4 * H], 2.0, -1.0,
                    mybir.AluOpType.mult, mybir.AluOpType.add,
                )
                gt_t[s] = gt

            def do_cupd():
                c = c_st[s]
                sifo, gt = sifo_t[s], gt_t[s]
                if t == 0:
                    nc.vector.tensor_mul(c[:], sifo[:, :, 0:H], gt[:])
                else:
                    fc = work.tile([128, CPS, H], f32, tag=f"fc{s}", name=f"fc{s}_{t}")
                    nc.gpsimd.tensor_mul(fc[:], sifo[:, :, H : 2 * H], c[:])
                    u = work.tile([128, CPS, H], f16, tag=f"u{s}", name=f"u{s}_{t}")
                    nc.vector.tensor_mul(u[:], sifo[:, :, 0:H], gt[:])
                    nc.vector.tensor_add(c[:], fc[:], u[:])

            def do_tanhc():
                tct = work.tile([128, CPS, H], f16, tag=f"tc{s}", name=f"tc{s}_{t}")
                nc.scalar.activation(tct[:], c_st[s][:], Tanh)
                tct_t[s] = tct

            def do_h():
                sifo = sifo_t[s]
                hh = work.tile([128, CPS, H], f16, tag=f"hh{s}", name=f"hh{s}_{t}")
                nc.vector.tensor_mul(hh[:], sifo[:, :, 2 * H : 3 * H], tct_t[s][:])
                if t == T - 1:
                    hh_last[s] = hh
                else:
                    hTn = work.tile([128, CPS * H], f16, tag=f"hT{s}",
                                    name=f"hT{s}_{t}")
                    nc.vector.transpose(hTn[:], hh[:])
                    hT[s] = hTn

            off = s / NSTREAM
            if s == 0:
                sched.append((t - 3 + 0.01, do_dma))
            sched.append((t + off + 0.00, do_mm))
            sched.append((t + off + 0.15, do_sigma))
            sched.append((t + off + 0.25, do_tanhg))
            sched.append((t + off + 0.40, do_cupd))
            sched.append((t + off + 0.60, do_tanhc))
            sched.append((t + off + 0.75, do_h))

        x_tiles = {}
        G_t = [None] * NSTREAM
        sifo_t = [None] * NSTREAM
        gt_t = [None] * NSTREAM
        tct_t = [None] * NSTREAM

        for t in range(T):
            for s in range(NSTREAM):
                emit_step(s, t)
        sched.sort(key=lambda kv: kv[0])
        for _, fn in sched:
            fn()

        # final projection via DVE: out[p,j,o] = sum_h hh[p,j,h]*W_out[o,h]
        tmp_po = work.tile([128, NSTREAM, CPS, 2, H], f32, name="tmp_po", tag="tmp_po")
        out_raw = const.tile([128, NCHUNK, 2], f32, name="out_raw")
        for s in range(NSTREAM):
            for o in range(2):
                nc.vector.tensor_mul(
                    tmp_po[:, s, :, o, :], hh_last[s][:], woutb_sb[:, :, o, :]
                )
        nc.vector.tensor_reduce(
            out_raw[:], tmp_po[:], axis=mybir.AxisListType.X, op=mybir.AluOpType.add
        )
        nc.vector.tensor_add(out_raw[:], out_raw[:], bout_sb[:])
        nc.default_dma_engine.dma_start(out=out_d.ap(), in_=out_raw[:])

    nc.compile()
    return nc


def _prep_inputs(x, W_ih, W_hh, b_ih, b_hh, W_out, b_out):
    # reorder pytorch gate rows [i,f,g,o] -> [i,f,o,g] so sigmoid gates are
    # contiguous in the free dim
    perm = np.concatenate(
        [np.arange(0, H), np.arange(H, 2 * H), np.arange(3 * H, 4 * H),
         np.arange(2 * H, 3 * H)]
    )
    Wih_r = np.asarray(W_ih)[perm]          # [128, 78]
    Whh_r = np.asarray(W_hh)[perm]          # [128, 32]
    bias_r = (np.asarray(b_ih) + np.asarray(b_hh))[perm]  # [128]

    wih = np.concatenate([Wih_r.T, bias_r[None, :]], axis=0)
    whh4 = np.tile(Whh_r.T, (4, 1))                                  # [128, 128]
    # tanh(x) = 2*sigmoid(2x)-1: fold the 2x into the g-gate columns
    wih[:, 3 * H :] *= 2.0
    whh4[:, 3 * H :] *= 2.0
    wih = wih.astype(np.float16)
    whh4 = whh4.astype(np.float16)
    woutb = np.tile(
        np.asarray(W_out)[None, None], (128, CPS, 1, 1)
    ).astype(np.float16)  # [128, CPS, 2, 32]
    bout = np.tile(np.asarray(b_out)[None, None, :], (128, NCHUNK, 1)).astype(
        np.float32
    )

    # x: [B, T, D] -> [T, D, B] fp16 with ones row appended -> [T, 79, B]
    xf = np.asarray(x).astype(np.float16)
    xT = np.empty((T, D + 1, B), np.float16)
    xT[:, :D, :] = xf.transpose(1, 2, 0)
    xT[:, D, :] = np.float16(1.0)

    in_maps = []
    for c in range(NCORES):
        in_maps.append(
            {
                "xT": np.ascontiguousarray(xT[:, :, BC * c : BC * (c + 1)]),
                "wih": wih,
                "whh4": whh4,
                "woutb": woutb,
                "bout": bout,
            }
        )
    return in_maps


def kernel(x, W_ih, W_hh, b_ih, b_hh, W_out, b_out, _trace=False):
    from concourse.bass_utils import run_bass_kernel_spmd

    if "nc" not in _CACHE:
        _CACHE["nc"] = _build_program()
    nc = _CACHE["nc"]

    in_maps = _prep_inputs(x, W_ih, W_hh, b_ih, b_hh, W_out, b_out)
    res = run_bass_kernel_spmd(nc, in_maps, list(range(NCORES)), trace=_trace)
    _CACHE["last_result"] = res

    out = np.empty((B, 2), np.float32)
    for c in range(NCORES):
        oc = res.results[c]["out"]          # [128, 4, 2]
        out[BC * c : BC * (c + 1)] = oc.transpose(1, 0, 2).reshape(BC, 2)
    return out


if __name__ == "__main__":
    rng = np.random.default_rng(0)
    ins = {
        "x": rng.standard_normal((B, T, D), dtype=np.float32),
        "W_ih": rng.uniform(-0.18, 0.18, (4 * H, D)).astype(np.float32),
        "W_hh": rng.uniform(-0.18, 0.18, (4 * H, H)).astype(np.float32),
        "b_ih": rng.uniform(-0.18, 0.18, (4 * H,)).astype(np.float32),
        "b_hh": rng.uniform(-0.18, 0.18, (4 * H,)).astype(np.float32),
        "W_out": rng.uniform(-0.18, 0.18, (2, H)).astype(np.float32),
        "b_out": rng.uniform(-0.18, 0.18, (2,)).astype(np.float32),
    }
    o = kernel(**ins)
    print(o.shape, o[:4])


# revision 12
# speedup vs baseline: 1.5451x; 1.4133x over previous
"""LSTM (B=4096, T=128, D=78, H=32) + Linear(32->2) on 8 NeuronCores.

Data-parallel over batch: 512 batch rows per core. On-core layout is
batch-on-partition (4 chunks of 128), gate values packed along the free
dimension so every elementwise op uses all 128 lanes. fp16 data / weights,
fp32 cell state and PSUM accumulation.
"""

import sys

sys.path.insert(0, "/opt/trn_rl_repo")

import numpy as np

B, T, D, H = 4096, 128, 78, 32
import os as _os0
T = int(_os0.environ.get("K_T", "128"))
NCORES = 8
BC = B // NCORES          # 512 batch per core
NCHUNK = BC // 128        # 4 chunks of 128 batch
import os as _os
NSTREAM = int(_os.environ.get("K_NSTREAM", "2"))  # independent pipelined streams
CPS = NCHUNK // NSTREAM   # chunks per stream

_CACHE = {}


def _build_program():
    import concourse.bacc as bacc
    import concourse.bass as bass
    import concourse.tile as tile
    from concourse import mybir
    from contextlib import ExitStack

    f16 = mybir.dt.float16
    f32 = mybir.dt.float32
    Sigmoid = mybir.ActivationFunctionType.Sigmoid
    Tanh = mybir.ActivationFunctionType.Tanh

    nc = bacc.Bacc("TRN2", target_bir_lowering=False, debug=False)

    xT_d = nc.dram_tensor("xT", [T, D + 1, BC], f16, kind="ExternalInput")
    wih_d = nc.dram_tensor("wih", [D + 1, 128], f16, kind="ExternalInput")
    whh4_d = nc.dram_tensor("whh4", [128, 128], f16, kind="ExternalInput")
    woutb_d = nc.dram_tensor("woutb", [128, CPS, 2, H], f16, kind="ExternalInput")
    bout_d = nc.dram_tensor("bout", [128, NCHUNK, 2], f32, kind="ExternalInput")
    out_d = nc.dram_tensor("out", [128, NCHUNK, 2], f32, kind="ExternalOutput")

    with ExitStack() as ctx:
        tc = ctx.enter_context(tile.TileContext(nc))
        const = ctx.enter_context(tc.tile_pool(name="const", bufs=1))
        xbufs = ctx.enter_context(tc.tile_pool(name="xbufs", bufs=8))
        psum = ctx.enter_context(tc.tile_pool(name="psum", bufs=2, space="PSUM"))
        work = ctx.enter_context(tc.tile_pool(name="work", bufs=3))
        state = ctx.enter_context(tc.tile_pool(name="state", bufs=1))

        wih_sb = const.tile([D + 1, 128], f16)
        nc.default_dma_engine.dma_start(out=wih_sb[:], in_=wih_d.ap())
        whh4_sb = const.tile([128, 128], f16)
        nc.default_dma_engine.dma_start(out=whh4_sb[:], in_=whh4_d.ap())
        woutb_sb = const.tile([128, CPS, 2, H], f16)
        nc.default_dma_engine.dma_start(out=woutb_sb[:], in_=woutb_d.ap())
        bout_sb = const.tile([128, NCHUNK, 2], f32)
        nc.default_dma_engine.dma_start(out=bout_sb[:], in_=bout_d.ap())

        # persistent per-stream cell state [128, CPS, 32] fp32
        c_st = [
            state.tile([128, CPS, H], f32, tag=f"c{s}", name=f"c{s}")
            for s in range(NSTREAM)
        ]
        hT = [None] * NSTREAM
        hh_last = [None] * NSTREAM

        xT_ap = xT_d.ap()

        # Emit instructions in an explicitly staggered order: the Tile list
        # scheduler follows emission order as priority, so stream 1 is
        # offset by half a step to pipeline the serial per-step chain.
        sched = []

        def emit_step(s, t):
            def do_dma():
                x_sb = xbufs.tile([D + 1, BC], f16, tag="x", name=f"x{t}")
                x_tiles[t] = x_sb
                nc.default_dma_engine.dma_start(out=x_sb[:], in_=xT_ap[t])

            def do_mm():
                G = psum.tile([128, CPS, 512], f32, tag=f"G{s}", name=f"G{s}_{t}")
                G_t[s] = G
                for jl in range(CPS):
                    j = s * CPS + jl
                    nc.tensor.matmul(
                        G[:, jl, 0:128],
                        x_tiles[t][:, 128 * j : 128 * (j + 1)],
                        wih_sb[:],
                        start=True,
                        stop=True,
                    )
                    if t > 0:
                        for a in range(4):
                            nc.tensor.matmul(
                                G[32 * a : 32 * (a + 1), jl, 0:128],
                                hT[s][32 * a : 32 * (a + 1), H * jl : H * (jl + 1)],
                                whh4_sb[32 * a : 32 * (a + 1), :],
                                start=False,
                                stop=False,
                                skip_group_check=True,
                                tile_position=(32 * a, 32 * a),
                            )

            def do_sigma():
                G = G_t[s]
                sifo = work.tile([128, CPS, 4 * H], f16, tag=f"sifo{s}",
                                 name=f"sifo{s}_{t}")
                nc.scalar.activation(sifo[:], G[:, :, 0 : 4 * H], Sigmoid)
                sifo_t[s] = sifo

            def do_tanhg():
                # g was pre-scaled by 2 in the weights: tanh(x) = 2*sigmoid(2x)-1
                gt = work.tile([128, CPS, H], f16, tag=f"g{s}", name=f"g{s}_{t}")
                nc.vector.tensor_scalar(
                    gt[:], sifo_t[s][:, :, 3 * H : 4 * H], 2.0, -1.0,
                    mybir.AluOpType.mult, mybir.AluOpType.add,
                )
                gt_t[s] = gt

            def do_cupd():
                c = c_st[s]
                sifo, gt = sifo_t[s], gt_t[s]
                if t == 0:
                    nc.vector.tensor_mul(c[:], sifo[:, :, 0:H], gt[:])
                else:
                    fc = work.tile([128, CPS, H], f32, tag=f"fc{s}", name=f"fc{s}_{t}")
                    nc.gpsimd.tensor_mul(fc[:], sifo[:, :, H : 2 * H], c[:])
                    u = work.tile([128, CPS, H], f16, tag=f"u{s}", name=f"u{s}_{t}")
                    nc.vector.tensor_mul(u[:], sifo[:, :, 0:H], gt[:])
                    nc.vector.tensor_add(c[:], fc[:], u[:])

            def do_tanhc():
                tct = work.tile([128, CPS, H], f16, tag=f"tc{s}", name=f"tc{s}_{t}")
                nc.scalar.activation(tct[:], c_st[s][:], Tanh)
                tct_t[s] = tct

            def do_h():
                sifo = sifo_t[s]
                hh = work.tile([128, CPS, H], f16, tag=f"hh{s}", name=f"hh{s}_{t}")
                nc.vector.tensor_mul(hh[:], sifo[:, :, 2 * H : 3 * H], tct_t[s][:])
                if t == T - 1:
                    hh_last[s] = hh
                else:
                    hTn = work.tile([128, CPS * H], f16, tag=f"hT{s}",
                                    name=f"hT{s}_{t}")
                    nc.vector.transpose(hTn[:], hh[:])
                    hT[s] = hTn

            off = s / NSTREAM
            if s == 0:
                sched.append((t - 3 + 0.01, do_dma))
            sched.append((t + off + 0.00, do_mm))
            sched.append((t + off + 0.15, do_sigma))
            sched.append((t + off + 0.25, do_tanhg))
            sched.append((t + off + 0.40, do_cupd))
            sched.append((t + off + 0.60, do_tanhc))
            sched.append((t + off + 0.75, do_h))

        x_tiles = {}
        G_t = [None] * NSTREAM
        sifo_t = [None] * NSTREAM
        gt_t = [None] * NSTREAM
        tct_t = [None] * NSTREAM

        for t in range(T):
            for s in range(NSTREAM):
                emit_step(s, t)
        sched.sort(key=lambda kv: kv[0])
        for _, fn in sched:
            fn()

        # final projection via DVE: out[p,j,o] = sum_h hh[p,j,h]*W_out[o,h]
        tmp_po = work.tile([128, NSTREAM, CPS, 2, H], f32, name="tmp_po", tag="tmp_po")
        out_raw = const.tile([128, NCHUNK, 2], f32, name="out_raw")
        for s in range(NSTREAM):
            for o in range(2):
                nc.vector.tensor_mul(
                    tmp_po[:, s, :, o, :], hh_last[s][:], woutb_sb[:, :, o, :]
                )
        nc.vector.tensor_reduce(
            out_raw[:], tmp_po[:], axis=mybir.AxisListType.X, op=mybir.AluOpType.add
        )
        nc.vector.tensor_add(out_raw[:], out_raw[:], bout_sb[:])
        nc.default_dma_engine.dma_start(out=out_d.ap(), in_=out_raw[:])

    nc.compile()
    return nc


def _prep_inputs(x, W_ih, W_hh, b_ih, b_hh, W_out, b_out):
    # reorder pytorch gate rows [i,f,g,o] -> [i,f,o,g] so sigmoid gates are
    # contiguous in the free dim
    perm = np.concatenate(
        [np.arange(0, H), np.arange(H, 2 * H), np.arange(3 * H, 4 * H),
         np.arange(2 * H, 3 * H)]
    )
    Wih_r = np.asarray(W_ih)[perm]          # [128, 78]
    Whh_r = np.asarray(W_hh)[perm]          # [128, 32]
    bias_r = (np.asarray(b_ih) + np.asarray(b_hh))[perm]  # [128]

    wih = np.concatenate([Wih_r.T, bias_r[None, :]], axis=0)
    whh4 = np.tile(Whh_r.T, (4, 1))                                  # [128, 128]
    # tanh(x) = 2*sigmoid(2x)-1: fold the 2x into the g-gate columns
    wih[:, 3 * H :] *= 2.0
    whh4[:, 3 * H :] *= 2.0
    wih = wih.astype(np.float16)
    whh4 = whh4.astype(np.float16)
    woutb = np.tile(
        np.asarray(W_out)[None, None], (128, CPS, 1, 1)
    ).astype(np.float16)  # [128, CPS, 2, 32]
    bout = np.tile(np.asarray(b_out)[None, None, :], (128, NCHUNK, 1)).astype(
        np.float32
    )

    # x: [B, T, D] -> [T, D, B] fp16 with ones row appended -> [T, 79, B]
    xf = np.asarray(x).astype(np.float16)[:, :T, :]
    xT = np.empty((T, D + 1, B), np.float16)
    xT[:, :D, :] = xf.transpose(1, 2, 0)
    xT[:, D, :] = np.float16(1.0)

    in_maps = []
    for c in range(NCORES):
        in_maps.append(
            {
                "xT": np.ascontiguousarray(xT[:, :, BC * c : BC * (c + 1)]),
                "wih": wih,
                "whh4": whh4,
                "woutb": woutb,
                "bout": bout,
            }
        )
    return in_maps


def kernel(x, W_ih, W_hh, b_ih, b_hh, W_out, b_out, _trace=False):
    from concourse.bass_utils import run_bass_kernel_spmd

    if "nc" not in _CACHE:
        _CACHE["nc"] = _build_program()
    nc = _CACHE["nc"]

    in_maps = _prep_inputs(x, W_ih, W_hh, b_ih, b_hh, W_out, b_out)
    res = run_bass_kernel_spmd(nc, in_maps, list(range(NCORES)), trace=_trace)
    _CACHE["last_result"] = res

    out = np.empty((B, 2), np.float32)
    for c in range(NCORES):
        oc = res.results[c]["out"]          # [128, 4, 2]
        out[BC * c : BC * (c + 1)] = oc.transpose(1, 0, 2).reshape(BC, 2)
    return out


if __name__ == "__main__":
    rng = np.random.default_rng(0)
    ins = {
        "x": rng.standard_normal((B, T, D), dtype=np.float32),
        "W_ih": rng.uniform(-0.18, 0.18, (4 * H, D)).astype(np.float32),
        "W_hh": rng.uniform(-0.18, 0.18, (4 * H, H)).astype(np.float32),
        "b_ih": rng.uniform(-0.18, 0.18, (4 * H,)).astype(np.float32),
        "b_hh": rng.uniform(-0.18, 0.18, (4 * H,)).astype(np.float32),
        "W_out": rng.uniform(-0.18, 0.18, (2, H)).astype(np.float32),
        "b_out": rng.uniform(-0.18, 0.18, (2,)).astype(np.float32),
    }
    o = kernel(**ins)
    print(o.shape, o[:4])
